# revision 1
# baseline (speedup 1.0000x reference)
"""Trainium2 Bass kernel for a binarized (XNOR-style) ResNet BasicBlock.

Reference semantics (per nn_BasicBlock_37228776522124):
    out = BN2(conv3x3(sign(BN1(conv3x3(sign(x), sign(w1)*a1))), sign(w2)*a2)) + x
with training-mode BN (batch stats over N,H,W) and per-out-channel
weight scale a_l = mean(|w_l|).

Key facts exploited:
  * conv inputs are exactly +-1 -> bf16 matmuls accumulate EXACT integers
    in fp32 PSUM (|z| <= 2304 < 2^24).
  * conv(sign(x), sign(w)*a) = a * conv(sign(x), sign(w)); a and BN fold
    into one per-channel affine s*z + b applied post-conv.
  * z is always even (sum of 256*k terms of +-1), so z/2 <= 1152 is stored
    exactly in fp16 (integers <= 2048 exact); conv1's z only feeds
    sign(z - mean), so it is stored as fp8 at z/16 (sign-safe rounding).
  * Data-parallel over batch (4 images/core on 8 cores); BN batch stats
    need one AllReduce of [128,4] fp32 per conv.

Self-contained: only needs /opt/trn_rl_repo (the Bass toolchain) + numpy.
"""

import os
import sys

for _p in ("/opt/trn_rl_repo",):
    if os.path.isdir(_p) and _p not in sys.path:
        sys.path.insert(0, _p)

import numpy as np

# Problem shapes (hardcoded per spec)
N_FULL, C, H, W = 32, 256, 56, 56
NCORES = 8
NPER = N_FULL // NCORES          # 4 images per core
SP = H * W                       # 3136
HP = H + 2                       # 58 (zero-padded)
SPP = HP * HP                    # 3364
NIB = C // 128                   # 2 input-channel blocks
NOB = C // 128                   # 2 output-channel blocks
NTAP = 9
NK = NTAP * NIB                  # 18 accumulation steps per output tile
RB = 7                           # row-blocks of 8 rows
RBW = 8 * W                      # 448 valid outputs per row-block
NMOV = 8 * HP                    # 464 moving columns (8 contiguous pad rows)
RBQ = NMOV + 2                   # 466 f32 <= one psum bank; tap tw writes a
                                 # contiguous 464 window at offset 2-tw; all
                                 # taps agree on q = r*58 + w + 2 for valid w
# BN normalizer = (images in the stats reduction) * SP, set in build_nc
EPS = 1e-5
KELEM = C * NTAP                 # 2304 weight elems per out channel

_nc_cache = {}


def build_nc(num_devices=NCORES):
    import concourse.bacc as bacc
    import concourse.tile as tile
    import concourse.mybir as mybir
    from concourse.masks import make_identity

    F32 = mybir.dt.float32
    F16 = mybir.dt.float16
    BF16 = mybir.dt.bfloat16
    ALU = mybir.AluOpType
    ACTF = mybir.ActivationFunctionType
    AX = mybir.AxisListType

    nc = bacc.Bacc(
        "TRN2", target_bir_lowering=False, debug=False,
        num_devices=num_devices,
    )

    x_t = nc.dram_tensor("x", [NPER, C, H, W], F32, kind="ExternalInput")
    w_t = [
        nc.dram_tensor("w1", [C, C, 3, 3], F32, kind="ExternalInput"),
        nc.dram_tensor("w2", [C, C, 3, 3], F32, kind="ExternalInput"),
    ]
    g_t = [
        nc.dram_tensor("gamma1", [C], F32, kind="ExternalInput"),
        nc.dram_tensor("gamma2", [C], F32, kind="ExternalInput"),
    ]
    b_t = [
        nc.dram_tensor("beta1", [C], F32, kind="ExternalInput"),
        nc.dram_tensor("beta2", [C], F32, kind="ExternalInput"),
    ]
    out_t = nc.dram_tensor("out", [NPER, C, H, W], F32, kind="ExternalOutput")

    x_ap = x_t.ap().rearrange("n c h w -> n c (h w)")      # [4, 256, 3136]
    out_ap = out_t.ap().rearrange("n c h w -> n c (h w)")
    rgroups = [list(range(num_devices))]
    M_TOTAL = float(num_devices * NPER * SP)
    # debug bisection: W < C1 < AR1 < C2 < FULL
    phase_lim = {"W": 0, "C1": 1, "AR1": 2, "C2": 3, "FULL": 9}[
        os.environ.get("KERNEL_PHASES", "FULL")]
    no_mm = bool(os.environ.get("KERNEL_NO_MM"))
    # fp8e4m3 DoubleRow matmuls: +-1 exact in fp8, 2 K-rows/cell -> ~2x PE
    use_fp8 = os.environ.get("KERNEL_FP8", "1") == "1"
    A8 = mybir.dt.float8e4
    PM = mybir.MatmulPerfMode
    # abuf block pitch: 2-col left margin (first-tap 466-wide matmul reads
    # from grid-2) + 3364 grid + tail pad; 3376 keeps fp8 pair-step 16B-aligned
    ABW = 3376
    GB = 2                          # grid base offset inside each block
    ABD = A8 if use_fp8 else BF16

    with tile.TileContext(nc) as tc:
        with (
            tc.tile_pool(name="consts", bufs=1) as pc,
            tc.tile_pool(name="dbl", bufs=2) as pd,
            tc.tile_pool(name="psum", bufs=8, space="PSUM") as pp,
            tc.tile_pool(name="dram", bufs=1, space="DRAM") as pdram,
        ):
            ident = pc.tile([128, 128], F32, name="ident", tag="ident")
            make_identity(nc, ident[:])
            epsap = pc.tile([128, 1], F32, name="epsap", tag="epsap")
            nc.vector.memset(epsap[:], EPS)

            # persistent stores: z/2 in fp16, [128, n, ob, 3136] flattened
            # z1 store only feeds sign(z - mean): fp8 at z/16 is
            # sign-safe (<=6% rel rounding, decisions are |z-mean|>~0.8).
            # z2 store feeds the output: fp16 at z/2 is exact (z even).
            zstore = [
                pc.tile([128, NPER * NOB * SP], A8 if l == 0 else F16,
                        name=f"z{l}", tag=f"z{l}")
                for l in range(2)
            ]
            zscale = [1.0 / 16.0, 0.5]
            wsign = [
                pc.tile([128, NK * NOB * 128], ABD, name=f"ws{l}", tag=f"ws{l}")
                for l in range(2)
            ]
            alphar = [pc.tile([128, NOB], F32, name=f"al{l}", tag=f"al{l}") for l in range(2)]
            sumc = [pc.tile([128, NOB * 28], F32, name=f"sc{l}", tag=f"sc{l}") for l in range(2)]
            sqc = [pc.tile([128, NOB * 28], F32, name=f"qc{l}", tag=f"qc{l}") for l in range(2)]
            statloc = [pc.tile([128, 4], F32, name=f"sl{l}", tag=f"sl{l}") for l in range(2)]
            statg = [pc.tile([128, 4], F32, name=f"sg{l}", tag=f"sg{l}") for l in range(2)]
            gb = [pc.tile([128, 2 * NOB], F32, name=f"gb{l}", tag=f"gb{l}") for l in range(2)]
            coef = [pc.tile([128, 2 * NOB], F32, name=f"cf{l}", tag=f"cf{l}") for l in range(2)]
            btmp = [pc.tile([128, 14], F32, name=f"bt{l}", tag=f"bt{l}") for l in range(2)]

            # dummy AllReduce at kernel start: absorbs the first-collective
            # latency (~60us) concurrently with conv1 so the real AR1 is fast
            ard_i = pdram.tile([128, 1], F32, name="ard_i", tag="ard_i")
            ard_o = pdram.tile([128, 1], F32, name="ard_o", tag="ard_o")
            nc.sync.dma_start(ard_i[:], g_t[0].ap()[0:128])
            nc.gpsimd.collective_compute(
                "AllReduce", ALU.add, replica_groups=rgroups,
                ins=[ard_i.opt()], outs=[ard_o.opt()],
            )
            # park the (unused) result in a spare btmp column so DCE keeps
            # it; gpsimd queue so the sync queue never waits on the AR
            nc.gpsimd.dma_start(btmp[0][:, 12:13], ard_o[:])

            # ---------------- weight prep ----------------
            def weight_prep(l):
                wd = w_t[l].ap().rearrange("o i h w -> o (i h w)")  # [256,2304]
                for ob in range(NOB):
                    wraw = pc.tile([128, KELEM], F32, name="wraw", tag="wraw",
                                   bufs=2)
                    nc.sync.dma_start(wraw[:], wd[ob * 128:(ob + 1) * 128, :])
                    # alpha_raw = sum |w| over (i,kh,kw), per out-channel row
                    nc.vector.tensor_reduce(
                        out=alphar[l][:, ob:ob + 1], in_=wraw[:],
                        axis=AX.X, op=ALU.add, apply_absolute_value=True,
                    )
                    wtap = wraw[:].rearrange("p (i t) -> p t i", t=NTAP)
                    for t in range(NTAP):
                        for ib in range(NIB):
                            if use_fp8:
                                # [p=i, (ob,t,pair=ib), m=o] for DoubleRow
                                kidx = (ob * NTAP + t) * 2 + ib
                            else:
                                kidx = ob * NK + t * NIB + ib
                            psT = pp.tile([128, RBW], F32, name="cps", tag="cps")
                            # transpose [o,i] -> [i,o] through the PE
                            nc.tensor.transpose(
                                psT[:, 0:128],
                                wtap[:, t, ib * 128:(ib + 1) * 128],
                                ident[:],
                            )
                            nc.scalar.activation(
                                out=wsign[l][:, kidx * 128:(kidx + 1) * 128],
                                in_=psT[:, 0:128], func=ACTF.Sign,
                            )
                # gamma/beta -> [128, col]
                for ob in range(NOB):
                    nc.sync.dma_start(
                        gb[l][:, ob:ob + 1],
                        g_t[l].ap()[ob * 128:(ob + 1) * 128],
                    )
                    nc.sync.dma_start(
                        gb[l][:, NOB + ob:NOB + ob + 1],
                        b_t[l].ap()[ob * 128:(ob + 1) * 128],
                    )

            weight_prep(0)

            # ---------------- one conv pass (shared for conv1/conv2) --------
            def conv_pass(l, act_fill, do_ar=True):
                """act_fill(n, abuf_ap) writes signed bf16 acts into the
                padded [128, NIB*SPP] buffer interior (ring already zero)."""
                for n in range(NPER):
                    abuf = pd.tile([128, NIB * ABW], ABD, name="abuf", tag="abuf")
                    for ib in range(NIB):
                        a58 = abuf[:, ib * ABW + GB:ib * ABW + GB + SPP
                                   ].rearrange("p (h w) -> p h w", w=HP)
                        nc.vector.memset(a58[:, 0:1, :], 0.0)
                        nc.vector.memset(a58[:, HP - 1:HP, :], 0.0)
                        nc.vector.memset(a58[:, :, 0:1], 0.0)
                        nc.vector.memset(a58[:, :, HP - 1:HP], 0.0)
                        # margins/tail: zero so junk psum columns stay finite
                        nc.vector.memset(abuf[:, ib * ABW:ib * ABW + GB], 0.0)
                        nc.vector.memset(
                            abuf[:, ib * ABW + GB + SPP:(ib + 1) * ABW], 0.0)
                    act_fill(n, abuf)
                    for ob in range(NOB):
                        ps = [pp.tile([128, RBQ], F32, name="cps", tag="cps")
                              for _ in range(RB)]
                        if use_fp8:
                            ab3 = abuf[:].rearrange(
                                "p (two s) -> p two s", two=NIB)
                            for t in range(NTAP):
                                th, tw = t // 3, t % 3
                                base = (ob * NTAP + t) * 2 * 128
                                lhsT = wsign[l][:, base:base + 256].rearrange(
                                    "p (two m) -> p two m", two=2)
                                for rb in range(RB):
                                    r0 = (rb * 8 + th) * HP
                                    if t == 0:
                                        # 466-wide: covers the whole psum
                                        # tile so has_written is uniform
                                        rhs = ab3[:, :, r0:r0 + RBQ]
                                        outap = ps[rb][:, 0:RBQ]
                                    else:
                                        rhs = ab3[:, :, GB + r0:GB + r0 + NMOV]
                                        outap = ps[rb][:, 2 - tw:2 - tw + NMOV]
                                    nc.tensor.matmul(
                                        outap, lhsT, rhs,
                                        start=(t == 0), stop=(t == NTAP - 1),
                                        perf_mode=PM.DoubleRow,
                                    )
                        else:
                            for k in range(NK):
                                t, ib = k // NIB, k % NIB
                                th, tw = t // 3, t % 3
                                kidx = ob * NK + k
                                af = abuf[:, ib * ABW:(ib + 1) * ABW]
                                lhsT = wsign[l][:, kidx * 128:(kidx + 1) * 128]
                                for rb in range(RB):
                                    r0 = (rb * 8 + th) * HP
                                    if no_mm and k > 0:
                                        continue
                                    if k == 0:
                                        rhs = af[:, r0:r0 + RBQ]
                                        outap = ps[rb][:, 0:RBQ]
                                    else:
                                        rhs = af[:, GB + r0:GB + r0 + NMOV]
                                        outap = ps[rb][:, 2 - tw:2 - tw + NMOV]
                                    nc.tensor.matmul(
                                        outap, lhsT, rhs,
                                        start=(k == 0),
                                        stop=(k == NK - 1) or no_mm,
                                    )
                        zs = zstore[l]
                        for rb in range(RB):
                            col = n * RB + rb
                            zsl = zs[:, ((n * NOB + ob) * SP + rb * RBW):
                                      ((n * NOB + ob) * SP + (rb + 1) * RBW)
                                      ].rearrange("p (h w) -> p h w", w=W)
                            qv = ps[rb][:, 2:2 + NMOV].rearrange(
                                "p (h w) -> p h w", w=HP)[:, :, 0:W]
                            # z/2 -> fp16 store on DVE; accum_out = sum(z/2)
                            nc.vector.tensor_scalar(
                                out=zsl, in0=qv,
                                scalar1=zscale[l], scalar2=None, op0=ALU.mult,
                                op1=ALU.add,
                                accum_out=sumc[l][:, ob * 28 + col:
                                                  ob * 28 + col + 1],
                            )
                            # scr = z^2 (dummy out); accum = sum(z^2)
                            scr = pd.tile([128, RBW], F32, name="scr", tag="scr")
                            nc.scalar.activation(
                                out=scr[:].rearrange("p (h w) -> p h w", w=W),
                                in_=qv, func=ACTF.Square,
                                accum_out=sqc[l][:, ob * 28 + col:
                                                 ob * 28 + col + 1],
                            )
                # local stats -> [sum_ob0, sum_ob1, sq_ob0, sq_ob1]
                for ob in range(NOB):
                    nc.vector.tensor_reduce(
                        out=statloc[l][:, ob:ob + 1],
                        in_=sumc[l][:, ob * 28:(ob + 1) * 28],
                        axis=AX.X, op=ALU.add,
                    )
                    nc.vector.tensor_reduce(
                        out=statloc[l][:, NOB + ob:NOB + ob + 1],
                        in_=sqc[l][:, ob * 28:(ob + 1) * 28],
                        axis=AX.X, op=ALU.add,
                    )
                if not do_ar:
                    return
                # AllReduce across cores (DRAM bounce). AR2's DMAs ride the
                # scalar queue so the sync queue can hold x prefetches
                # without blocking it (in-order queues).
                dma_eng = nc.sync if l == 0 else nc.scalar
                arin = pdram.tile([128, 4], F32, name=f"ari{l}", tag=f"ari{l}")
                arout = pdram.tile([128, 4], F32, name=f"aro{l}", tag=f"aro{l}")
                dma_eng.dma_start(arin[:], statloc[l][:])
                nc.gpsimd.collective_compute(
                    "AllReduce", ALU.add, replica_groups=rgroups,
                    ins=[arin.opt()], outs=[arout.opt()],
                )
                dma_eng.dma_start(statg[l][:], arout[:])
                # BN fold: coef = [2s | beta - s*mean] per ob column
                tmp = btmp[l]
                for ob in range(NOB):
                    mean = tmp[:, 0 + ob * 6:1 + ob * 6]
                    e2 = tmp[:, 1 + ob * 6:2 + ob * 6]
                    var = tmp[:, 2 + ob * 6:3 + ob * 6]
                    alp = tmp[:, 3 + ob * 6:4 + ob * 6]
                    tt = tmp[:, 4 + ob * 6:5 + ob * 6]
                    std = tmp[:, 5 + ob * 6:6 + ob * 6]
                    # mean = 2*sum(z/2)/M ; E[z^2] = sumsq/M ; var = E - mean^2
                    nc.vector.tensor_scalar_mul(
                        mean, statg[l][:, ob:ob + 1], 1.0 / (zscale[l] * M_TOTAL))
                    nc.vector.tensor_scalar_mul(
                        e2, statg[l][:, NOB + ob:NOB + ob + 1], 1.0 / M_TOTAL)
                    nc.vector.tensor_mul(var, mean, mean)
                    nc.vector.tensor_sub(var, e2, var)
                    nc.vector.tensor_scalar_mul(
                        alp, alphar[l][:, ob:ob + 1], 1.0 / KELEM)
                    nc.vector.tensor_mul(tt, alp, alp)
                    nc.vector.tensor_mul(tt, tt, var)
                    # std = sqrt(alpha^2 var + eps) ; inv = 1/std (accurate)
                    nc.scalar.activation(std, tt, ACTF.Sqrt, bias=epsap[:])
                    nc.vector.reciprocal(tt, std)
                    nc.vector.tensor_mul(tt, tt, alp)             # alpha*inv
                    nc.vector.tensor_mul(tt, tt, gb[l][:, ob:ob + 1])  # *gamma
                    # sp = s / zscale (applied to the scaled z store)
                    nc.vector.tensor_scalar_mul(
                        coef[l][:, ob:ob + 1], tt, 1.0 / zscale[l])
                    # bb = beta - s*mean
                    nc.vector.tensor_mul(tt, tt, mean)
                    nc.vector.tensor_sub(
                        coef[l][:, NOB + ob:NOB + ob + 1],
                        gb[l][:, NOB + ob:NOB + ob + 1], tt)

            # ---------------- conv1: acts = sign(x) ----------------
            def fill1(n, abuf):
                for ib in range(NIB):
                    xin = pd.tile([128, SP], F32, name="xin", tag="xin")
                    nc.sync.dma_start(
                        xin[:], x_ap[n, ib * 128:(ib + 1) * 128, :])
                    a58 = abuf[:, ib * ABW + GB:ib * ABW + GB + SPP].rearrange(
                        "p (h w) -> p h w", w=HP)
                    xv = xin[:].rearrange("p (h w) -> p h w", w=W)
                    nc.scalar.activation(
                        out=a58[:, 1:H + 1, 1:W + 1], in_=xv, func=ACTF.Sign)


            if phase_lim >= 1:
                conv_pass(0, fill1, do_ar=(phase_lim >= 2))

            # conv2 weight prep here: its PE transposes overlap the AR1 wait
            weight_prep(1)

            # ---------------- conv2: acts = sign(s1*z1 + b1) ----------------
            def fill2(n, abuf):
                for ib in range(NIB):
                    a58 = abuf[:, ib * ABW + GB:ib * ABW + GB + SPP].rearrange(
                        "p (h w) -> p h w", w=HP)
                    zv = zstore[0][:, (n * NOB + ib) * SP:
                                   (n * NOB + ib + 1) * SP].rearrange(
                        "p (h w) -> p h w", w=W)
                    nc.scalar.activation(
                        out=a58[:, 1:H + 1, 1:W + 1], in_=zv, func=ACTF.Sign,
                        scale=coef[0][:, ib:ib + 1],
                        bias=coef[0][:, NOB + ib:NOB + ib + 1],
                    )

            # x prefetch for the finalize: issued on the sync queue ahead of
            # conv2 in program order, 4 dedicated slots; slot 5+ waits on
            # finalize consumption (sync queue has nothing downstream that
            # the ARs need, so this never deadlocks)
            xfins = []
            for k in range(8 if phase_lim >= 9 else 0):
                xf = pd.tile([128, SP], F32, name="xfin", tag="xfin",
                             bufs=4)
                xfins.append(xf)
            for k in range(4 if phase_lim >= 9 else 0):
                n, ob = k // NOB, k % NOB
                nc.sync.dma_start(
                    xfins[k][:], x_ap[n, ob * 128:(ob + 1) * 128, :])

            if phase_lim >= 3:
                conv_pass(1, fill2, do_ar=(phase_lim >= 9))

            if phase_lim < 9:
                # debug: dump something touching live tiles into out
                dbg = pd.tile([128, SP], F32, name="dbg", tag="xin")
                if phase_lim >= 1:
                    nc.vector.tensor_copy(dbg[:], zstore[0][:, 0:SP])
                else:
                    nc.vector.tensor_copy(dbg[:], wsign[0][:, 0:SP])
                nc.sync.dma_start(out_ap[0, 0:128, :], dbg[:])

            # ---------------- finalize: out = s2*z2 + b2 + x ----------------
            for n in range(NPER if phase_lim >= 9 else 0):
                for ob in range(NOB):
                    k = n * NOB + ob
                    # alternate recycled tags -> 4 effective t1 slots
                    t1 = pd.tile([128, SP], F32, name="t1",
                                 tag=("abuf" if k % 2 == 0 else "xin"))
                    # t1 = z2' * (2 s2) + b2 on ACT; +x on DVE (pipelines)
                    nc.scalar.activation(
                        out=t1[:],
                        in_=zstore[1][:, (n * NOB + ob) * SP:
                                      (n * NOB + ob + 1) * SP],
                        func=ACTF.Identity,
                        scale=coef[1][:, ob:ob + 1],
                        bias=coef[1][:, NOB + ob:NOB + ob + 1],
                    )
                    # late x-loads ride the scalar queue just behind the
                    # affine whose add frees their slot (2-wide loads)
                    if k < 4:
                        kl = k + 4
                        nc.scalar.dma_start(
                            xfins[kl][:],
                            x_ap[kl // NOB, (kl % NOB) * 128:
                                 (kl % NOB + 1) * 128, :])
                    nc.vector.tensor_add(t1[:], t1[:], xfins[k][:])
                    dma_eng = nc.gpsimd if k % 2 == 0 else nc.sync
                    dma_eng.dma_start(
                        out_ap[n, ob * 128:(ob + 1) * 128, :], t1[:])

    nc.compile()
    return nc


def _get_nc(num_devices=NCORES):
    if num_devices not in _nc_cache:
        _nc_cache[num_devices] = build_nc(num_devices)
    return _nc_cache[num_devices]


def kernel(**inputs):
    from concourse.bass_utils import run_bass_kernel_spmd

    nc = _get_nc(NCORES)
    x = np.ascontiguousarray(np.asarray(inputs["x"], dtype=np.float32))
    shared = {
        k: np.ascontiguousarray(np.asarray(inputs[k], dtype=np.float32))
        for k in ("w1", "gamma1", "beta1", "w2", "gamma2", "beta2")
    }
    in_maps = [
        {"x": x[c * NPER:(c + 1) * NPER], **shared} for c in range(NCORES)
    ]
    res = run_bass_kernel_spmd(nc, in_maps, core_ids=list(range(NCORES)))
    out = np.concatenate([r["out"] for r in res.results], axis=0)
    return out.astype(np.float32)

